# revision 22
# baseline (speedup 1.0000x reference)
"""Trainium2 Bass kernel for nn_AttentionBlock (B=8, S=1024, D=1024, H=16).

Strategy: pure data-parallel over batch -- each of the 8 NeuronCores gets one
batch element and runs the full attention block on it. No collectives.

Math (per batch element b):
  qkv = x @ W_in.T + b_in ; q,k,v per head ; s = (q @ k.T) * scale
  alpha = softmax(s) * m ; alpha /= sum(alpha) ; out = alpha @ v ; out @ W_out.T
The softmax normalizer cancels against the multiplier renormalization:
  final_alpha = (exp(s) * m) / sum_k (exp(s) * m)
so we never compute softmax: one exp per score, one elementwise multiply,
one row-sum (via ones-columns in the attn@v stationary), one divide.
|s| <= ~6 for this data so exp needs no max-subtraction.

Head-pair structure: heads 2j and 2j+1 land on opposite partition halves of
the same QKV projection tile, so their score matmuls (K=64 contraction each)
run CONCURRENTLY on disjoint PE row groups with no operand replication.
Per (pair j, k-tile ka):
  sA[k,q] = k_even.T @ q_even   (PE rows 0-63)   -> psum A [128,1024]
  sB[k,q] = k_odd.T @ q_odd     (PE rows 64-127) -> psum B [128,1024]
  eX = exp(sX) (ScalarE), tX = eX * m[ka] (DVE)
  U_even[qn] += [v_e|1].T @ tA[:,qn]        (streamed per k-tile)
  U_odd[qn]  += [..|1|v_o].T @ tB[:,qn]     (burst at pair end, tB retained)
The vaug region per pair is [v_e(64) | 1 | 1 | v_o(64)] (130 cols): the odd
head's stationary is cols 2:130 so its U lands on psum partitions 63(den),
64-127(v) and the normalized output is written by DVE straight into attnT
partitions 64-127 (DVE lanes cannot cross partitions).  The even/odd split
of the attn@v stream (even per-k-tile, odd end-burst) keeps the U pool at 2
PSUM banks: scores 2x2 + filler fills 2 + U 2 = 8 banks exactly.
QK/V projection tiles are interleaved into phase 3 as PE filler (their own
1-buf psum pool) while ScalarE runs the exps.
"""

import os
import numpy as np
import ml_dtypes

BF16 = ml_dtypes.bfloat16

B, S, D = 8, 1024, 1024
H, HD = 16, 64
NPAIR = H // 2
P = 128
NQT = S // 512       # 2 q-column halves (512 = fp32 psum bank)
NKT = S // P         # 8 k tiles
NDI = D // P         # 8 contraction tiles
SCALE = 1.0 / np.sqrt(HD)

_CACHE = {}


def _build_program(with_bias=False):
    import concourse.mybir as mybir
    import concourse.tile as tile
    from concourse import bacc

    fp32 = mybir.dt.float32
    bf16 = mybir.dt.bfloat16
    AFT = mybir.ActivationFunctionType

    nc = bacc.Bacc(None)

    xT_d = nc.declare_dram_parameter("xT", [D, S], bf16, isOutput=False)
    wqkT_d = nc.declare_dram_parameter("wqkT", [D, 2 * D], bf16, isOutput=False)
    wvT_d = nc.declare_dram_parameter("wvT", [D, D], bf16, isOutput=False)
    mT_d = nc.declare_dram_parameter("mT", [S, S], bf16, isOutput=False)
    woutT_d = nc.declare_dram_parameter("woutT", [D, D], bf16, isOutput=False)
    if with_bias:
        bqk_d = nc.declare_dram_parameter("bqk", [1, 2 * D], bf16, isOutput=False)
        bv_d = nc.declare_dram_parameter("bv", [1, D], bf16, isOutput=False)
        bout_d = nc.declare_dram_parameter("bout", [1, D], bf16, isOutput=False)
    out_d = nc.declare_dram_parameter("out", [S, D], fp32, isOutput=True)

    with tile.TileContext(nc) as tc:
        with (
            tc.tile_pool(name="const", bufs=1) as cpool,
            tc.tile_pool(name="weights", bufs=1) as wpool,
            tc.tile_pool(name="acts", bufs=1) as apool,
            tc.tile_pool(name="epool", bufs=2) as ep,
            tc.tile_pool(name="tpool", bufs=2) as tp,
            tc.tile_pool(name="uevac", bufs=2) as uev,
            tc.tile_pool(name="rpool", bufs=1) as rp,
            tc.tile_pool(name="oevac", bufs=1) as oev,
            tc.tile_pool(name="ps", bufs=1, space="PSUM") as ps_pool,
            tc.tile_pool(name="fps", bufs=1, space="PSUM") as f_pool,
            tc.tile_pool(name="us", bufs=2, space="PSUM") as u_pool,
        ):
            # ---- constants ----
            warm_w = cpool.tile([P, P], bf16)
            nc.gpsimd.memset(warm_w, 0.5)
            if with_bias:
                ones_1x512 = cpool.tile([1, 512], bf16)
                nc.gpsimd.memset(ones_1x512, 1.0)
                ones_1x128 = ones_1x512[:, :P]
                bqk_sb = cpool.tile([1, 2 * D], bf16)
                nc.sync.dma_start(bqk_sb[:], bqk_d[:])
                bv_sb = cpool.tile([1, D], bf16)
                nc.sync.dma_start(bv_sb[:], bv_d[:])
                bout_sb = cpool.tile([1, D], bf16)
                nc.sync.dma_start(bout_sb[:], bout_d[:])

            # warm the exp table before phase 3 needs it
            warm = cpool.tile([1, 1], fp32)
            nc.gpsimd.memset(warm, 0.0)
            warm2 = cpool.tile([1, 1], fp32)
            nc.scalar.activation(warm2[:], warm[:], AFT.Exp)

            # ---- input loads; emission order = landing order.  xT+wqkT
            # first (lead-in qk fills), then wvT, mT, woutT. ----
            xT_sb = wpool.tile([P, NDI, S], bf16)
            wqkT_sb = wpool.tile([P, NDI, 2 * D], bf16)
            wvT_sb = wpool.tile([P, NDI, D], bf16)
            mT_sb = wpool.tile([P, NKT, S], bf16)
            woutT_sb = wpool.tile([P, NDI, D], bf16)
            xT_r = xT_d.rearrange("(o p) f -> p o f", p=P)
            wqkT_r = wqkT_d.rearrange("(o p) f -> p o f", p=P)
            wvT_r = wvT_d.rearrange("(o p) f -> p o f", p=P)
            mT_r = mT_d.rearrange("(o p) f -> p o f", p=P)
            for di in range(NDI):
                nc.sync.dma_start(xT_sb[:, di], xT_r[:, di])
                nc.sync.dma_start(wqkT_sb[:, di], wqkT_r[:, di])
            for di in range(NDI):
                nc.sync.dma_start(wvT_sb[:, di], wvT_r[:, di])
            for st in range(NKT):
                nc.sync.dma_start(mT_sb[:, st], mT_r[:, st])
            nc.sync.dma_start(woutT_sb[:], woutT_d.rearrange("(o p) f -> p o f", p=P))

            qkT_sb = apool.tile([P, 16, S], bf16)   # do-tiles 0-7 = qT, 8-15 = kT
            # per (k-tile, head): [v(64) | 1]
            vaug_sb = apool.tile([P, NKT, H, HD + 1], bf16)
            nc.gpsimd.memset(vaug_sb[:, :, :, HD:HD + 1], 1.0)
            attnT_sb = apool.tile([P, NDI, S], bf16)

            # ---- HAM warmup: ~3us of dummy matmuls while input DMA lands ----
            wps = ps_pool.tile([P, S], fp32, tag="ps")
            for i in range(28):
                nc.tensor.matmul(
                    wps[:, (i % 4) * P:(i % 4 + 1) * P], warm_w[:], warm_w[:],
                    start=True, stop=True,
                )

            def fill_qk(dot, pool):
                # one column-tile of qkT = Wqk @ x.T (+ bias); evac on DVE
                ps = pool.tile([P, S], fp32, tag="ps" if pool is ps_pool else "f")
                for di in range(NDI):
                    lhsT = wqkT_sb[:, di, dot * P:(dot + 1) * P]
                    for qn in range(NQT):
                        nc.tensor.matmul(
                            ps[:, qn * 512:(qn + 1) * 512],
                            lhsT,
                            xT_sb[:, di, qn * 512:(qn + 1) * 512],
                            start=(di == 0), stop=(not with_bias and di == NDI - 1),
                        )
                if with_bias:
                    for qn in range(NQT):
                        nc.tensor.matmul(
                            ps[:, qn * 512:(qn + 1) * 512],
                            bqk_sb[:, dot * P:(dot + 1) * P],
                            ones_1x512[:],
                            start=False, stop=True,
                        )
                nc.vector.tensor_copy(out=qkT_sb[:, dot, :], in_=ps[:])

            def fill_v(st, pool):
                # v[seq-tile st, dv] = x @ Wv.T (+ bv), packed into vaug
                ps = pool.tile([P, S], fp32, tag="ps" if pool is ps_pool else "f")
                for di in range(NDI):
                    lhsT = xT_sb[:, di, st * P:(st + 1) * P]
                    for dn in range(NQT):
                        nc.tensor.matmul(
                            ps[:, dn * 512:(dn + 1) * 512],
                            lhsT,
                            wvT_sb[:, di, dn * 512:(dn + 1) * 512],
                            start=(di == 0), stop=(not with_bias and di == NDI - 1),
                        )
                if with_bias:
                    for dn in range(NQT):
                        nc.tensor.matmul(
                            ps[:, dn * 512:(dn + 1) * 512],
                            ones_1x128[:],
                            bv_sb[:, dn * 512:(dn + 1) * 512],
                            start=False, stop=True,
                        )
                # psum free dim u = 64*h + e
                src = ps[:].rearrange("p (h e) -> p h e", e=HD)
                nc.vector.tensor_copy(out=vaug_sb[:, st, :, 0:HD], in_=src[:])

            # ---- lead-in: just enough for pair 0 to start (score pool is
            # free here, so these four fills double-buffer across pools) ----
            fill_qk(0, ps_pool)      # q heads 0,1
            fill_qk(8, f_pool)       # k heads 0,1
            fill_v(0, ps_pool)
            fill_v(1, f_pool)

            def normalize(u_ps, qn, j, even):
                # evac U psum -> SBUF fp32 (frees the bank), reciprocal of
                # the den row straight from psum, partition-broadcast, then
                # multiply into attnT.  v at psum p0-63, den at p64 for both
                # parities; odd heads bounce via DMA (DVE lanes cannot cross
                # partitions to reach attnT rows 64-127).
                sl = slice(qn * 512, (qn + 1) * 512)
                uv = uev.tile([P, 512], fp32, tag="ue")
                rr = rp.tile([P, 512], fp32, tag="r")
                Rb = rp.tile([P, 512], fp32, tag="R")
                nc.vector.tensor_copy(out=uv[0:HD + 1, :], in_=u_ps[0:HD + 1, :])
                # den sits at partition 64; partition_broadcast reads its
                # source from partition 0, so bounce the row down via DMA
                dsb = rp.tile([1, 512], fp32, tag="d", bufs=1)
                nc.sync.dma_start(dsb[0:1, :], uv[HD:HD + 1, :])
                nc.vector.reciprocal_approx_fast(
                    out=rr[0:1, :], in_=dsb[0:1, :])
                nc.gpsimd.partition_broadcast(Rb[0:HD, :], rr[0:1, :])
                if even:
                    nc.vector.tensor_mul(
                        out=attnT_sb[0:HD, j, sl], in0=uv[0:HD, :], in1=Rb[0:HD, :])
                else:
                    tmp = rp.tile([HD, 512], bf16, tag="tmp", bufs=2)
                    nc.vector.tensor_mul(
                        out=tmp[:], in0=uv[0:HD, :], in1=Rb[0:HD, :])
                    nc.sync.dma_start(attnT_sb[HD:P, j, sl], tmp[:])

            # ---- phase 3: attention per head pair; remaining fills are
            # emitted interleaved as PE filler while ScalarE runs exps ----
            for j in range(NPAIR):
                qt = qkT_sb[:, j, :]        # q: even head rows 0-63, odd 64-127
                kt = qkT_sb[:, 8 + j, :]
                Ue = [u_pool.tile([P, 512], fp32, tag="u", name=f"Ue{j}_{qn}")
                      for qn in range(NQT)]
                tbs = []
                for ka in range(NKT):
                    # filler fills (dedicated 1-buf psum pool; executed
                    # lazily by the scheduler in PE gaps)
                    if j == 0 and ka < 6:
                        fill_v(ka + 2, f_pool)
                    elif j == 0 and ka == 6:
                        fill_qk(1, f_pool)
                    elif j == 0 and ka == 7:
                        fill_qk(9, f_pool)
                    elif 1 <= j <= 6 and ka == 4:
                        fill_qk(j + 1, f_pool)
                    elif 1 <= j <= 6 and ka == 6:
                        fill_qk(j + 9, f_pool)
                    ks = slice(ka * P, (ka + 1) * P)
                    # one 4-bank psum tile for BOTH heads: the single exp
                    # below frees it in one event, so the next kt's even/odd
                    # score matmuls become ready TOGETHER and the scheduler
                    # keeps each disjoint-row-group pair adjacent (concurrent)
                    sab = ps_pool.tile([P, 2 * S], fp32, tag="ps")
                    for qn in range(NQT):
                        sl = slice(qn * 512, (qn + 1) * 512)
                        slb = slice(S + qn * 512, S + (qn + 1) * 512)
                        nc.tensor.matmul(
                            sab[:, sl], kt[0:HD, ks], qt[0:HD, sl],
                            start=True, stop=True,
                        )
                        nc.tensor.matmul(
                            sab[:, slb], kt[HD:P, ks], qt[HD:P, sl],
                            start=True, stop=True,
                        )
                    eab = ep.tile([P, 2 * S], bf16, tag="e")
                    nc.scalar.activation(eab[:], sab[:], AFT.Exp)
                    ta = tp.tile([P, S], bf16, tag="ta", bufs=2)
                    nc.vector.tensor_mul(
                        out=ta[:], in0=eab[:, 0:S], in1=mT_sb[:, ka, :])
                    # tb is retained until the pair-end odd burst (8 live)
                    tb = tp.tile([P, S], bf16, tag="tb", bufs=8)
                    nc.vector.tensor_mul(
                        out=tb[:], in0=eab[:, S:2 * S], in1=mT_sb[:, ka, :])
                    tbs.append(tb)
                    # even head attn@v streams per k-tile
                    for qn in range(NQT):
                        sl = slice(qn * 512, (qn + 1) * 512)
                        nc.tensor.matmul(
                            Ue[qn][0:HD + 1, :], vaug_sb[:, ka, 2 * j, 0:HD + 1],
                            ta[:, sl],
                            start=(ka == 0), stop=(ka == NKT - 1),
                        )
                # free the even-U banks quickly, then the odd-head burst
                for qn in range(NQT):
                    normalize(Ue[qn], qn, j, even=True)
                for qn in range(NQT):
                    Uo = u_pool.tile([P, 512], fp32, tag="u", name=f"Uo{j}_{qn}")
                    sl = slice(qn * 512, (qn + 1) * 512)
                    for ka in range(NKT):
                        nc.tensor.matmul(
                            Uo[0:HD + 1, :], vaug_sb[:, ka, 2 * j + 1, 0:HD + 1],
                            tbs[ka][:, sl],
                            start=(ka == 0), stop=(ka == NKT - 1),
                        )
                    normalize(Uo, qn, j, even=False)

            # ---- phase 4: final[q, do] = attnT.T @ WoutT (+ bout);
            # alternate the two psum pools for 2-deep pipelining ----
            for qt_i in range(NKT):
                pool = ps_pool if qt_i % 2 == 0 else f_pool
                ps = pool.tile([P, S], fp32,
                               tag="ps" if qt_i % 2 == 0 else "f")
                for di in range(NDI):
                    lhsT = attnT_sb[:, di, qt_i * P:(qt_i + 1) * P]
                    for dn in range(NQT):
                        nc.tensor.matmul(
                            ps[:, dn * 512:(dn + 1) * 512],
                            lhsT,
                            woutT_sb[:, di, dn * 512:(dn + 1) * 512],
                            start=(di == 0), stop=(not with_bias and di == NDI - 1),
                        )
                if with_bias:
                    for dn in range(NQT):
                        nc.tensor.matmul(
                            ps[:, dn * 512:(dn + 1) * 512],
                            ones_1x128[:],
                            bout_sb[:, dn * 512:(dn + 1) * 512],
                            start=False, stop=True,
                        )
                for dn in range(NQT):
                    sl = slice(dn * 512, (dn + 1) * 512)
                    o = oev.tile([P, 512], fp32, tag="o", bufs=2)
                    nc.scalar.copy(out=o[:], in_=ps[:, sl])
                    nc.sync.dma_start(
                        out_d[qt_i * P:(qt_i + 1) * P, sl], o[:])

    return nc


def _prep_inputs(x, multipliers, W_in, b_in, W_out, b_out):
    x = np.asarray(x, dtype=np.float32)
    multipliers = np.asarray(multipliers, dtype=np.float32)
    W_in = np.asarray(W_in, dtype=np.float32)
    b_in = np.asarray(b_in, dtype=np.float32)
    W_out = np.asarray(W_out, dtype=np.float32)
    b_out = np.asarray(b_out, dtype=np.float32)

    wqk = W_in[:2 * D].copy()
    wqk[:D] *= SCALE                      # fold 1/sqrt(hd) into q projection
    wqkT = np.ascontiguousarray(wqk.T).astype(BF16)
    wvT = np.ascontiguousarray(W_in[2 * D:].T).astype(BF16)
    woutT = np.ascontiguousarray(W_out.T).astype(BF16)
    with_bias = bool(np.any(b_in) or np.any(b_out))
    bias_maps = {}
    if with_bias:
        bqk = b_in[:2 * D].copy()
        bqk[:D] *= SCALE
        bias_maps = {
            "bqk": bqk.reshape(1, -1).astype(BF16),
            "bv": b_in[2 * D:].reshape(1, -1).astype(BF16),
            "bout": b_out.reshape(1, -1).astype(BF16),
        }

    in_maps = []
    for b in range(B):
        xT = np.ascontiguousarray(x[b].T).astype(BF16)
        mT = np.ascontiguousarray(multipliers[b].T).astype(BF16)
        in_maps.append({
            "xT": xT, "wqkT": wqkT, "wvT": wvT, "mT": mT,
            "woutT": woutT, **bias_maps,
        })
    return in_maps, with_bias


LAST_RESULT = None  # BassKernelResults of the most recent run (for test harness)


def _enable_axon_trace():
    """Register the NTFF profile hook that this image's antenv lacks."""
    import sys as _sys
    try:
        import antenv.axon_hooks  # noqa: F401
        return True
    except ImportError:
        pass
    try:
        import types
        import antenv
        from trn_agent_boot.trn_boot import _ntff_profile_via_ctypes
        hook = _ntff_profile_via_ctypes("/opt/axon/libaxon_pjrt.so")
        if hook is None:
            return False
        mod = types.ModuleType("antenv.axon_hooks")
        state = {"hook": hook}
        mod.get_axon_ntff_profile_hook = lambda: state["hook"]
        mod.set_axon_ntff_profile_hook = lambda h: state.__setitem__("hook", h)
        _sys.modules["antenv.axon_hooks"] = mod
        antenv.axon_hooks = mod
        # keep profile artifacts local; no network bucket in this container
        import concourse.bass_utils as bu
        bu.upload_artifacts = lambda tmpdir: tmpdir
        return True
    except Exception:
        return False


def kernel(x, multipliers, W_in, b_in, W_out, b_out):
    global LAST_RESULT
    from concourse.bass_utils import run_bass_kernel_spmd

    in_maps, with_bias = _prep_inputs(x, multipliers, W_in, b_in, W_out, b_out)
    key = ("nc", with_bias)
    if key not in _CACHE:
        nc = _build_program(with_bias=with_bias)
        if not nc.is_finalized():
            nc.finalize()  # runs Bacc legalization (reg alloc, wait splitting)
        _CACHE[key] = nc
    nc = _CACHE[key]
    trace = os.environ.get("BASS_KERNEL_TRACE", "0") == "1"
    if trace:
        trace = _enable_axon_trace()

    def _run(do_trace):
        return run_bass_kernel_spmd(
            nc, in_maps, core_ids=list(range(B)), trace=do_trace,
            tmpdir=os.environ.get("BASS_KERNEL_TMPDIR") if do_trace else None,
        )

    res = None
    last_exc = None
    for attempt in range(3):
        try:
            res = _run(trace and attempt == 0)
            break
        except Exception as exc:  # e.g. device left wedged by a prior process
            last_exc = exc
            try:
                import jax
                jax.clear_caches()
                jax.clear_backends()
            except Exception:
                pass
    if res is None:
        raise last_exc
    LAST_RESULT = res
    out = np.stack([res.results[i]["out"] for i in range(B)]).astype(np.float32)
    return out


# revision 26
# speedup vs baseline: 1.0294x; 1.0294x over previous
"""Trainium2 Bass kernel for nn_AttentionBlock (B=8, S=1024, D=1024, H=16).

Strategy: pure data-parallel over batch -- each of the 8 NeuronCores gets one
batch element and runs the full attention block on it. No collectives.

Math (per batch element b):
  qkv = x @ W_in.T + b_in ; q,k,v per head ; s = (q @ k.T) * scale
  alpha = softmax(s) * m ; alpha /= sum(alpha) ; out = alpha @ v ; out @ W_out.T
The softmax normalizer cancels against the multiplier renormalization:
  final_alpha = (exp(s) * m) / sum_k (exp(s) * m)
so we never compute softmax: one exp per score, one elementwise multiply,
one row-sum (via ones-columns in the attn@v stationary), one divide.
|s| <= ~6 for this data so exp needs no max-subtraction.

Head-pair structure: heads 2j and 2j+1 land on opposite partition halves of
the same QKV projection tile, so their score matmuls (K=64 contraction each)
run CONCURRENTLY on disjoint PE row groups with no operand replication.
Per (pair j, k-tile ka):
  sA[k,q] = k_even.T @ q_even   (PE rows 0-63)   -> psum A [128,1024]
  sB[k,q] = k_odd.T @ q_odd     (PE rows 64-127) -> psum B [128,1024]
  eX = exp(sX) (ScalarE), tX = eX * m[ka] (DVE)
  U_even[qn] += [v_e|1].T @ tA[:,qn]        (streamed per k-tile)
  U_odd[qn]  += [..|1|v_o].T @ tB[:,qn]     (burst at pair end, tB retained)
The vaug region per pair is [v_e(64) | 1 | 1 | v_o(64)] (130 cols): the odd
head's stationary is cols 2:130 so its U lands on psum partitions 63(den),
64-127(v) and the normalized output is written by DVE straight into attnT
partitions 64-127 (DVE lanes cannot cross partitions).  The even/odd split
of the attn@v stream (even per-k-tile, odd end-burst) keeps the U pool at 2
PSUM banks: scores 2x2 + filler fills 2 + U 2 = 8 banks exactly.
QK/V projection tiles are interleaved into phase 3 as PE filler (their own
1-buf psum pool) while ScalarE runs the exps.
"""

import os
import numpy as np
import ml_dtypes

BF16 = ml_dtypes.bfloat16

B, S, D = 8, 1024, 1024
H, HD = 16, 64
NPAIR = H // 2
P = 128
NQT = S // 512       # 2 q-column halves (512 = fp32 psum bank)
NKT = S // P         # 8 k tiles
NDI = D // P         # 8 contraction tiles
SCALE = 1.0 / np.sqrt(HD)

_CACHE = {}


def _build_program(with_bias=False):
    import concourse.mybir as mybir
    import concourse.tile as tile
    from concourse import bacc

    fp32 = mybir.dt.float32
    bf16 = mybir.dt.bfloat16
    AFT = mybir.ActivationFunctionType

    nc = bacc.Bacc(None)

    xT_d = nc.declare_dram_parameter("xT", [D, S], bf16, isOutput=False)
    wqkT_d = nc.declare_dram_parameter("wqkT", [D, 2 * D], bf16, isOutput=False)
    wvT_d = nc.declare_dram_parameter("wvT", [D, D], bf16, isOutput=False)
    mT_d = nc.declare_dram_parameter("mT", [S, S], bf16, isOutput=False)
    woutT_d = nc.declare_dram_parameter("woutT", [D, D], bf16, isOutput=False)
    if with_bias:
        bqk_d = nc.declare_dram_parameter("bqk", [1, 2 * D], bf16, isOutput=False)
        bv_d = nc.declare_dram_parameter("bv", [1, D], bf16, isOutput=False)
        bout_d = nc.declare_dram_parameter("bout", [1, D], bf16, isOutput=False)
    out_d = nc.declare_dram_parameter("out", [S, D], fp32, isOutput=True)

    with tile.TileContext(nc) as tc:
        with (
            tc.tile_pool(name="const", bufs=1) as cpool,
            tc.tile_pool(name="weights", bufs=1) as wpool,
            tc.tile_pool(name="acts", bufs=1) as apool,
            tc.tile_pool(name="epool", bufs=2) as ep,
            tc.tile_pool(name="tpool", bufs=2) as tp,
            tc.tile_pool(name="rpool", bufs=1) as rp,
            tc.tile_pool(name="oevac", bufs=1) as oev,
            tc.tile_pool(name="ps", bufs=2, space="PSUM") as ps_pool,
            tc.tile_pool(name="fps", bufs=1, space="PSUM") as f_pool,
            tc.tile_pool(name="us", bufs=2, space="PSUM") as u_pool,
        ):
            # ---- constants ----
            warm_w = cpool.tile([P, P], bf16)
            nc.gpsimd.memset(warm_w, 0.5)
            if with_bias:
                ones_1x512 = cpool.tile([1, 512], bf16)
                nc.gpsimd.memset(ones_1x512, 1.0)
                ones_1x128 = ones_1x512[:, :P]
                bqk_sb = cpool.tile([1, 2 * D], bf16)
                nc.sync.dma_start(bqk_sb[:], bqk_d[:])
                bv_sb = cpool.tile([1, D], bf16)
                nc.sync.dma_start(bv_sb[:], bv_d[:])
                bout_sb = cpool.tile([1, D], bf16)
                nc.sync.dma_start(bout_sb[:], bout_d[:])

            # warm the exp table before phase 3 needs it
            warm = cpool.tile([1, 1], fp32)
            nc.gpsimd.memset(warm, 0.0)
            warm2 = cpool.tile([1, 1], fp32)
            nc.scalar.activation(warm2[:], warm[:], AFT.Exp)

            # ---- input loads; emission order = landing order.  xT+wqkT
            # first (lead-in qk fills), then wvT, mT, woutT. ----
            xT_sb = wpool.tile([P, NDI, S], bf16)
            wqkT_sb = wpool.tile([P, NDI, 2 * D], bf16)
            wvT_sb = wpool.tile([P, NDI, D], bf16)
            mT_sb = wpool.tile([P, NKT, S], bf16)
            woutT_sb = wpool.tile([P, NDI, D], bf16)
            xT_r = xT_d.rearrange("(o p) f -> p o f", p=P)
            wqkT_r = wqkT_d.rearrange("(o p) f -> p o f", p=P)
            wvT_r = wvT_d.rearrange("(o p) f -> p o f", p=P)
            mT_r = mT_d.rearrange("(o p) f -> p o f", p=P)
            for di in range(NDI):
                nc.sync.dma_start(xT_sb[:, di], xT_r[:, di])
                nc.sync.dma_start(wqkT_sb[:, di], wqkT_r[:, di])
            for di in range(NDI):
                nc.sync.dma_start(wvT_sb[:, di], wvT_r[:, di])
            for st in range(NKT):
                nc.sync.dma_start(mT_sb[:, st], mT_r[:, st])
            nc.sync.dma_start(woutT_sb[:], woutT_d.rearrange("(o p) f -> p o f", p=P))

            qkT_sb = apool.tile([P, 16, S], bf16)   # do-tiles 0-7 = qT, 8-15 = kT
            # per (k-tile, head): [v(64) | 1]
            vaug_sb = apool.tile([P, NKT, H, HD + 1], bf16)
            nc.gpsimd.memset(vaug_sb[:, :, :, HD:HD + 1], 1.0)
            attnT_sb = apool.tile([P, NDI, S], bf16)

            # ---- HAM warmup: ~3us of dummy matmuls while input DMA lands ----
            wps = ps_pool.tile([P, S], fp32, tag="ps")
            for i in range(28):
                nc.tensor.matmul(
                    wps[:, (i % 4) * P:(i % 4 + 1) * P], warm_w[:], warm_w[:],
                    start=True, stop=True,
                )

            def fill_qk(dot, pool):
                # one column-tile of qkT = Wqk @ x.T (+ bias); evac on DVE
                ps = pool.tile([P, S], fp32, tag="ps" if pool is ps_pool else "f")
                for di in range(NDI):
                    lhsT = wqkT_sb[:, di, dot * P:(dot + 1) * P]
                    for qn in range(NQT):
                        nc.tensor.matmul(
                            ps[:, qn * 512:(qn + 1) * 512],
                            lhsT,
                            xT_sb[:, di, qn * 512:(qn + 1) * 512],
                            start=(di == 0), stop=(not with_bias and di == NDI - 1),
                        )
                if with_bias:
                    for qn in range(NQT):
                        nc.tensor.matmul(
                            ps[:, qn * 512:(qn + 1) * 512],
                            bqk_sb[:, dot * P:(dot + 1) * P],
                            ones_1x512[:],
                            start=False, stop=True,
                        )
                nc.vector.tensor_copy(out=qkT_sb[:, dot, :], in_=ps[:])

            def fill_v(st, pool):
                # v[seq-tile st, dv] = x @ Wv.T (+ bv), packed into vaug
                ps = pool.tile([P, S], fp32, tag="ps" if pool is ps_pool else "f")
                for di in range(NDI):
                    lhsT = xT_sb[:, di, st * P:(st + 1) * P]
                    for dn in range(NQT):
                        nc.tensor.matmul(
                            ps[:, dn * 512:(dn + 1) * 512],
                            lhsT,
                            wvT_sb[:, di, dn * 512:(dn + 1) * 512],
                            start=(di == 0), stop=(not with_bias and di == NDI - 1),
                        )
                if with_bias:
                    for dn in range(NQT):
                        nc.tensor.matmul(
                            ps[:, dn * 512:(dn + 1) * 512],
                            ones_1x128[:],
                            bv_sb[:, dn * 512:(dn + 1) * 512],
                            start=False, stop=True,
                        )
                # psum free dim u = 64*h + e
                src = ps[:].rearrange("p (h e) -> p h e", e=HD)
                nc.vector.tensor_copy(out=vaug_sb[:, st, :, 0:HD], in_=src[:])

            # ---- lead-in: just enough for pair 0 to start (score pool is
            # free here, so these four fills double-buffer across pools) ----
            fill_qk(0, ps_pool)      # q heads 0,1
            fill_qk(8, f_pool)       # k heads 0,1
            fill_v(0, ps_pool)
            fill_v(1, f_pool)

            def normalize(u_ps, qn, j, even):
                # den row (psum p64) -> partition 0 via DMA (the gpsimd
                # broadcast reads its source from partition 0), reciprocal,
                # partition-broadcast, then multiply straight from psum into
                # attnT.  Odd heads bounce via DMA (DVE lanes cannot cross
                # partitions to reach attnT rows 64-127).
                sl = slice(qn * 512, (qn + 1) * 512)
                dn = rp.tile([HD + 1, 512], fp32, tag="dn", bufs=2)
                nc.vector.tensor_copy(out=dn[HD:HD + 1, :], in_=u_ps[HD:HD + 1, :])
                dsb = rp.tile([1, 512], fp32, tag="d", bufs=2)
                nc.sync.dma_start(dsb[0:1, :], dn[HD:HD + 1, :])
                rr = rp.tile([1, 512], fp32, tag="r", bufs=2)
                nc.vector.reciprocal_approx_fast(
                    out=rr[0:1, :], in_=dsb[0:1, :])
                Rb = rp.tile([HD, 512], fp32, tag="R", bufs=2)
                nc.gpsimd.partition_broadcast(Rb[0:HD, :], rr[0:1, :])
                if even:
                    nc.vector.tensor_mul(
                        out=attnT_sb[0:HD, j, sl], in0=u_ps[0:HD, :],
                        in1=Rb[0:HD, :])
                else:
                    tmp = rp.tile([HD, 512], bf16, tag="tmp", bufs=2)
                    nc.vector.tensor_mul(
                        out=tmp[:], in0=u_ps[0:HD, :], in1=Rb[0:HD, :])
                    nc.sync.dma_start(attnT_sb[HD:P, j, sl], tmp[:])

            # ---- phase 3: attention per head pair; remaining fills are
            # emitted interleaved as PE filler while ScalarE runs exps ----
            for j in range(NPAIR):
                qt = qkT_sb[:, j, :]        # q: even head rows 0-63, odd 64-127
                kt = qkT_sb[:, 8 + j, :]
                Ue = [u_pool.tile([P, 512], fp32, tag="u", name=f"Ue{j}_{qn}")
                      for qn in range(NQT)]
                tbs = []
                for ka in range(NKT):
                    # filler fills (dedicated 1-buf psum pool; executed
                    # lazily by the scheduler in PE gaps).  Pair 7 refills
                    # two dead tiles purely to keep the PE/HAM clock warm
                    # into phase 4.
                    if j == 0 and ka < 6:
                        fill_v(ka + 2, f_pool)
                    elif j == 0 and ka == 6:
                        fill_qk(1, f_pool)
                    elif j == 0 and ka == 7:
                        fill_qk(9, f_pool)
                    elif 1 <= j <= 6 and ka == 4:
                        fill_qk(j + 1, f_pool)
                    elif 1 <= j <= 6 and ka == 6:
                        fill_qk(j + 9, f_pool)
                    elif j == 7 and ka in (2, 5):
                        fill_qk(1 if ka == 2 else 9, f_pool)
                    ks = slice(ka * P, (ka + 1) * P)
                    # one 2-bank psum tile per qn-half holding BOTH heads:
                    # the single exp over it frees both heads' slots in one
                    # event, so the next kt's even/odd score matmuls become
                    # ready TOGETHER and run concurrently on disjoint PE
                    # row groups, while bufs=2 keeps ScalarE densely fed.
                    ts = []
                    for qn in range(NQT):
                        sl = slice(qn * 512, (qn + 1) * 512)
                        s2 = ps_pool.tile([P, S], fp32, tag="ps",
                                          name=f"s{j}_{ka}_{qn}")
                        nc.tensor.matmul(
                            s2[:, 0:512], kt[0:HD, ks], qt[0:HD, sl],
                            start=True, stop=True,
                        )
                        nc.tensor.matmul(
                            s2[:, 512:1024], kt[HD:P, ks], qt[HD:P, sl],
                            start=True, stop=True,
                        )
                        e2 = ep.tile([P, S], bf16, tag="e")
                        nc.scalar.activation(e2[:], s2[:], AFT.Exp)
                        ta = tp.tile([P, 512], bf16, tag="ta", bufs=3)
                        nc.vector.tensor_mul(
                            out=ta[:], in0=e2[:, 0:512], in1=mT_sb[:, ka, sl])
                        # to is retained until the pair-end odd burst
                        to = tp.tile([P, 512], bf16, tag="tb", bufs=17)
                        nc.vector.tensor_mul(
                            out=to[:], in0=e2[:, 512:1024], in1=mT_sb[:, ka, sl])
                        ts.append((ta, to))
                        # even head attn@v streams per k-tile
                        nc.tensor.matmul(
                            Ue[qn][0:HD + 1, :], vaug_sb[:, ka, 2 * j, 0:HD + 1],
                            ta[:],
                            start=(ka == 0), stop=(ka == NKT - 1),
                        )
                    tbs.append((ts[0][1], ts[1][1]))
                # free the even-U banks quickly, then the odd-head burst
                for qn in range(NQT):
                    normalize(Ue[qn], qn, j, even=True)
                for qn in range(NQT):
                    Uo = u_pool.tile([P, 512], fp32, tag="u", name=f"Uo{j}_{qn}")
                    for ka in range(NKT):
                        nc.tensor.matmul(
                            Uo[0:HD + 1, :], vaug_sb[:, ka, 2 * j + 1, 0:HD + 1],
                            tbs[ka][qn][:],
                            start=(ka == 0), stop=(ka == NKT - 1),
                        )
                    normalize(Uo, qn, j, even=False)

            # ---- phase 4: final[q, do] = attnT.T @ WoutT (+ bout);
            # alternate the two psum pools for 2-deep pipelining ----
            for qt_i in range(NKT):
                pool = ps_pool if qt_i % 2 == 0 else f_pool
                ps = pool.tile([P, S], fp32,
                               tag="ps" if qt_i % 2 == 0 else "f")
                for di in range(NDI):
                    lhsT = attnT_sb[:, di, qt_i * P:(qt_i + 1) * P]
                    for dn in range(NQT):
                        nc.tensor.matmul(
                            ps[:, dn * 512:(dn + 1) * 512],
                            lhsT,
                            woutT_sb[:, di, dn * 512:(dn + 1) * 512],
                            start=(di == 0), stop=(not with_bias and di == NDI - 1),
                        )
                if with_bias:
                    for dn in range(NQT):
                        nc.tensor.matmul(
                            ps[:, dn * 512:(dn + 1) * 512],
                            ones_1x128[:],
                            bout_sb[:, dn * 512:(dn + 1) * 512],
                            start=False, stop=True,
                        )
                for dn in range(NQT):
                    sl = slice(dn * 512, (dn + 1) * 512)
                    o = oev.tile([P, 512], fp32, tag="o", bufs=2)
                    nc.scalar.copy(out=o[:], in_=ps[:, sl])
                    nc.sync.dma_start(
                        out_d[qt_i * P:(qt_i + 1) * P, sl], o[:])

    return nc


def _prep_inputs(x, multipliers, W_in, b_in, W_out, b_out):
    x = np.asarray(x, dtype=np.float32)
    multipliers = np.asarray(multipliers, dtype=np.float32)
    W_in = np.asarray(W_in, dtype=np.float32)
    b_in = np.asarray(b_in, dtype=np.float32)
    W_out = np.asarray(W_out, dtype=np.float32)
    b_out = np.asarray(b_out, dtype=np.float32)

    wqk = W_in[:2 * D].copy()
    wqk[:D] *= SCALE                      # fold 1/sqrt(hd) into q projection
    wqkT = np.ascontiguousarray(wqk.T).astype(BF16)
    wvT = np.ascontiguousarray(W_in[2 * D:].T).astype(BF16)
    woutT = np.ascontiguousarray(W_out.T).astype(BF16)
    with_bias = bool(np.any(b_in) or np.any(b_out))
    bias_maps = {}
    if with_bias:
        bqk = b_in[:2 * D].copy()
        bqk[:D] *= SCALE
        bias_maps = {
            "bqk": bqk.reshape(1, -1).astype(BF16),
            "bv": b_in[2 * D:].reshape(1, -1).astype(BF16),
            "bout": b_out.reshape(1, -1).astype(BF16),
        }

    in_maps = []
    for b in range(B):
        xT = np.ascontiguousarray(x[b].T).astype(BF16)
        mT = np.ascontiguousarray(multipliers[b].T).astype(BF16)
        in_maps.append({
            "xT": xT, "wqkT": wqkT, "wvT": wvT, "mT": mT,
            "woutT": woutT, **bias_maps,
        })
    return in_maps, with_bias


LAST_RESULT = None  # BassKernelResults of the most recent run (for test harness)


def _enable_axon_trace():
    """Register the NTFF profile hook that this image's antenv lacks."""
    import sys as _sys
    try:
        import antenv.axon_hooks  # noqa: F401
        return True
    except ImportError:
        pass
    try:
        import types
        import antenv
        from trn_agent_boot.trn_boot import _ntff_profile_via_ctypes
        hook = _ntff_profile_via_ctypes("/opt/axon/libaxon_pjrt.so")
        if hook is None:
            return False
        mod = types.ModuleType("antenv.axon_hooks")
        state = {"hook": hook}
        mod.get_axon_ntff_profile_hook = lambda: state["hook"]
        mod.set_axon_ntff_profile_hook = lambda h: state.__setitem__("hook", h)
        _sys.modules["antenv.axon_hooks"] = mod
        antenv.axon_hooks = mod
        # keep profile artifacts local; no network bucket in this container
        import concourse.bass_utils as bu
        bu.upload_artifacts = lambda tmpdir: tmpdir
        return True
    except Exception:
        return False


def kernel(x, multipliers, W_in, b_in, W_out, b_out):
    global LAST_RESULT
    from concourse.bass_utils import run_bass_kernel_spmd

    in_maps, with_bias = _prep_inputs(x, multipliers, W_in, b_in, W_out, b_out)
    key = ("nc", with_bias)
    if key not in _CACHE:
        nc = _build_program(with_bias=with_bias)
        if not nc.is_finalized():
            nc.finalize()  # runs Bacc legalization (reg alloc, wait splitting)
        _CACHE[key] = nc
    nc = _CACHE[key]
    trace = os.environ.get("BASS_KERNEL_TRACE", "0") == "1"
    if trace:
        trace = _enable_axon_trace()

    def _run(do_trace):
        return run_bass_kernel_spmd(
            nc, in_maps, core_ids=list(range(B)), trace=do_trace,
            tmpdir=os.environ.get("BASS_KERNEL_TMPDIR") if do_trace else None,
        )

    res = None
    last_exc = None
    for attempt in range(3):
        try:
            res = _run(trace and attempt == 0)
            break
        except Exception as exc:  # e.g. device left wedged by a prior process
            last_exc = exc
            try:
                import jax
                jax.clear_caches()
                jax.clear_backends()
            except Exception:
                pass
    if res is None:
        raise last_exc
    LAST_RESULT = res
    out = np.stack([res.results[i]["out"] for i in range(B)]).astype(np.float32)
    return out
